# revision 13
# baseline (speedup 1.0000x reference)
"""Trainium2 Bass kernel for MinimalLightningAttention2.

Strategy (8 NeuronCores, SPMD, no collectives):
  core c -> batch b = c // 4, head group g = c % 4 (heads 4g..4g+3).
  Each core computes, fully fused on-chip:
    qkv projection (its 4 heads' columns of Wqkv)
    chunked lightning-attention scan (L=128 chunks, per-head decay state S)
    row-parallel partial of the output projection (its 4 heads' rows of Wout)
  Host sums the 4 partial outputs per batch and adds bout.

Layouts on device (per core):
  xT   [c, it, kt, n]  host-pre-transposed/packed bf16 x; every DMA granule
                       is contiguous per partition (kt-major within a span)
  q,k  [d,   n]  (lhsT = Wq/Wk tile, rhs = xT)
  v    [n, h*d]  (lhsT = xT tile,    rhs = Wv)
  attn output oT [e, i] per head -> directly the lhsT of the Wout matmul.
All matmuls in bf16 (PSUM accumulation fp32); decay masks applied in fp32
during PSUM eviction; decay state S kept in bf16 with a fused per-head
(S*blockdecay + update) scalar_tensor_tensor.

Perf notes (measured on trn2 via NTFF traces):
  - kernel is PE-streaming-bound (~95% TensorMatrix busy); remaining time
    is startup (time-to-first-MM), the output-DMA tail, and the serial
    S-chain bubbles in the last span.
  - startup: ~100 warmup matmuls on the identity tile run while the first
    DMAs land, so the PE's HAM clock-gate is already at full rate when
    real work starts; first x/wq k-tiles are 128KB each, in need-order on
    the sync HWDGE ring; span-0 q/k run kt-outer across 4 PSUM banks so
    each arriving granule immediately feeds 4 matmuls.
  - outputs are bf16 and go per-chunk on the sync HWDGE ring: hardware
    descriptor generation (the gpsimd SWDGE takes ~4.2us/chunk to build
    descriptors, which serialized into a ~11us drain tail).
  - PSUM banks: qkv 3 + attn-sc 1 + attn-o 2 + outproj 2 = 8 (all used;
    span-0's 4th projection bank is borrowed from the idle outproj pool).
"""

import math

import numpy as np
import ml_dtypes

B, N, C = 2, 4096, 2048
H_TOT = 16
HD = 128          # head dim
H = 4             # heads per core
L = 128           # attention chunk length
NCH = N // L      # 32 chunks
KT = C // 128     # 16 contraction tiles for the projections
NSPAN = 512       # tokens per outer iteration
NIT = N // NSPAN  # 8 outer iterations
P = 128

BF16 = ml_dtypes.bfloat16

_CACHE = {}


def _build():
    """Build + compile the SPMD Bass program (same program on all 8 cores)."""
    from contextlib import ExitStack

    import concourse.bass as bass
    import concourse.tile as tile
    from concourse import bacc, mybir

    DT = mybir.dt.bfloat16
    F32 = mybir.dt.float32

    nc = bacc.Bacc(
        "TRN2",
        target_bir_lowering=False,
        debug=False,
        enable_asserts=False,
        num_devices=8,
    )

    # host-packed transpose of x: xd[c, it, kt, n] = x[it*512+n, kt*128+c]
    # (per-partition contiguous within a span: kt-major, 1KB per kt)
    xd = nc.dram_tensor("x", [P, NIT, KT, NSPAN], DT, kind="ExternalInput").ap()
    # host-packed: [c, kt*512 + col] (col = head*128 + d), fully contiguous rows
    wqd = nc.dram_tensor("wq", [P, KT * 512], DT, kind="ExternalInput").ap()
    wkd = nc.dram_tensor("wk", [P, KT * 512], DT, kind="ExternalInput").ap()
    wvd = nc.dram_tensor("wv", [P, KT * 512], DT, kind="ExternalInput").ap()
    # host-packed: [d, h*2048 + outc]
    wod = nc.dram_tensor("wo", [P, H * C], DT, kind="ExternalInput").ap()
    masktd = nc.dram_tensor("maskt", [P, H * L], F32, kind="ExternalInput").ap()
    qdecd = nc.dram_tensor("qdec", [P, H * NSPAN], F32, kind="ExternalInput").ap()
    kdecvd = nc.dram_tensor("kdecv", [P, H * HD], F32, kind="ExternalInput").ap()
    bdfcd = nc.dram_tensor("bdfc", [P, H], F32, kind="ExternalInput").ap()
    bqkd = nc.dram_tensor("bqk", [P, 2 * H], F32, kind="ExternalInput").ap()
    bvfd = nc.dram_tensor("bvf", [P, H * HD], F32, kind="ExternalInput").ap()
    outd = nc.dram_tensor("out", [N, C], DT, kind="ExternalOutput").ap()

    mult = mybir.AluOpType.mult
    add = mybir.AluOpType.add

    with tile.TileContext(nc) as tc:
        with ExitStack() as ctx:
            const = ctx.enter_context(tc.tile_pool(name="const", bufs=1))
            xt0_pool = ctx.enter_context(tc.tile_pool(name="xt0", bufs=1))
            xt_pool = ctx.enter_context(tc.tile_pool(name="xt", bufs=2))
            qk_pool = ctx.enter_context(tc.tile_pool(name="qk", bufs=2))
            sc_pool = ctx.enter_context(tc.tile_pool(name="sc", bufs=4))
            ob_pool = ctx.enter_context(tc.tile_pool(name="ob", bufs=3))
            outb_pool = ctx.enter_context(tc.tile_pool(name="outb", bufs=2))
            qkv_ps = ctx.enter_context(tc.tile_pool(name="qkvps", bufs=3, space="PSUM"))
            attn_sc_ps = ctx.enter_context(tc.tile_pool(name="attnscps", bufs=1, space="PSUM"))
            attn_o_ps = ctx.enter_context(tc.tile_pool(name="attnops", bufs=2, space="PSUM"))
            out_ps = ctx.enter_context(tc.tile_pool(name="outps", bufs=2, space="PSUM"))

            # identity + PE warmup first: ~100 no-dep matmuls run while the
            # first DMAs land so the HAM clock-gate reaches full rate before
            # real work (and the PE never sits >3.4us idle before it).
            ident = const.tile([P, P], DT)
            from concourse.masks import make_identity
            make_identity(nc, ident)
            warm_ps = attn_o_ps.tile([P, 512], F32, tag="o", name="warm")
            for _ in range(48):
                nc.tensor.matmul(warm_ps[:, 0:P], lhsT=ident[:], rhs=ident[:],
                                 start=True, stop=True)

            # ---- constants / weights resident in SBUF ----
            # All big startup loads on the ONE sync ring, in need-order: each
            # DGE ring is a FIFO and rings fair-share HBM at packet
            # granularity. The first x k-tile and first wq k-tile are 128KB
            # each and interleave 1-ktile-wise so the PE can start after
            # ~256KB of DMA. Small decay/bias constants go on the gpsimd
            # (SWDGE) ring which flows earlier anyway.
            # startup needs few triggers: each sync dma_start is a ~650ns
            # DIRECT2D sequencer instruction, FIFO-serialized.
            xk0 = xt0_pool.tile([P, NSPAN], DT, name="xk0")          # span-0 kt0
            nc.sync.dma_start(xk0[:], xd[:, 0, 0, :])
            wqk0 = const.tile([P, 512], DT, name="wqk0")             # wq kt0
            nc.sync.dma_start(wqk0[:], wqd[:, 0:512])
            xkr = xt0_pool.tile([P, 3, NSPAN], DT, name="xkr")       # span-0 kt1-3
            nc.sync.dma_start(xkr[:], xd[:, 0, 1:4, :])
            wqkr = const.tile([P, 3 * 512], DT, name="wqkr")         # wq kt1-3
            nc.sync.dma_start(wqkr[:], wqd[:, 512:2048])
            xq0 = [None]  # span-0 x, quarters 1..3
            wq_q = [None]  # wq quarters 1..3
            for qq in range(1, 4):
                xq = xt_pool.tile([P, 4, NSPAN], DT, tag=f"xtq{qq}", name=f"xt0q{qq}")
                nc.sync.dma_start(xq[:], xd[:, 0, qq * 4:(qq + 1) * 4, :])
                xq0.append(xq)
                wt = const.tile([P, 4 * 512], DT, tag=f"wqq{qq}", name=f"wqq{qq}")
                nc.sync.dma_start(wt[:], wqd[:, qq * 2048:(qq + 1) * 2048])
                wq_q.append(wt)
            # small decay/bias constants (needed from the first evictions on,
            # ~26us in); everything on the one sync ring in need-order — the
            # gpsimd SWDGE is left with no DMA work at all.
            bqk_sb = const.tile([P, 2 * H], F32)
            nc.sync.dma_start(bqk_sb[:], bqkd[:])
            qdec_sb = const.tile([P, H * NSPAN], F32)
            nc.sync.dma_start(qdec_sb[:], qdecd[:])
            kdecv_sb = const.tile([P, H * HD], F32)
            nc.sync.dma_start(kdecv_sb[:], kdecvd[:])
            bdfc_sb = const.tile([P, H], F32)
            nc.sync.dma_start(bdfc_sb[:], bdfcd[:])
            bvf_sb = const.tile([P, H * HD], F32)
            nc.sync.dma_start(bvf_sb[:], bvfd[:])
            maskt_sb = const.tile([P, H * L], F32)
            nc.sync.dma_start(maskt_sb[:], masktd[:])
            wk_q = []
            wv_q = []
            for qq in range(4):
                t = const.tile([P, 4 * 512], DT, tag=f"wkq{qq}", name=f"wkq{qq}")
                nc.sync.dma_start(t[:], wkd[:, qq * 2048:(qq + 1) * 2048])
                wk_q.append(t)
            for qq in range(4):
                t = const.tile([P, 4 * 512], DT, tag=f"wvq{qq}", name=f"wvq{qq}")
                nc.sync.dma_start(t[:], wvd[:, qq * 2048:(qq + 1) * 2048])
                wv_q.append(t)
            wo_sb = const.tile([P, H * C], DT)
            nc.sync.dma_start(wo_sb[:], wod[:])

            def wq_slice(kt, h):
                if kt == 0:
                    return wqk0[:, h * HD:(h + 1) * HD]
                if kt < 4:
                    return wqkr[:, (kt - 1) * 512 + h * HD:(kt - 1) * 512 + (h + 1) * HD]
                return wq_q[kt // 4][:, (kt % 4) * 512 + h * HD:(kt % 4) * 512 + (h + 1) * HD]

            def wk_slice(kt, h):
                return wk_q[kt // 4][:, (kt % 4) * 512 + h * HD:(kt % 4) * 512 + (h + 1) * HD]

            # per-head decay state S [d, e], 4 heads side by side, bf16
            S_bf = const.tile([P, H * HD], DT)
            nc.vector.memset(S_bf[:], 0.0)

            xt_tiles = [None] * NIT
            for it in range(NIT):
                n0 = it * NSPAN
                # prefetch next span's xT (host-packed, four contiguous quarters)
                if it + 1 < NIT:
                    quarters = []
                    for qq in range(4):
                        xq = xt_pool.tile([P, 4, NSPAN], DT, tag=f"xtq{qq}", name=f"xtq{qq}_{it + 1}")
                        nc.sync.dma_start(xq[:], xd[:, it + 1, qq * 4:(qq + 1) * 4, :])
                        quarters.append(xq)
                    xt_tiles[it + 1] = quarters

                if it == 0:
                    def xts(kt):
                        if kt == 0:
                            return xk0[:]
                        if kt < 4:
                            return xkr[:, kt - 1, :]
                        return xq0[kt // 4][:, kt % 4, :]
                else:
                    xtab = xt_tiles[it]

                    def xts(kt, xtab=xtab):
                        return xtab[kt // 4][:, kt % 4, :]

                # ---- qkv projection for the span ----
                q_raw = qk_pool.tile([P, H * NSPAN], DT, tag="q_raw")
                q_dec = qk_pool.tile([P, H * NSPAN], DT, tag="q_dec")
                k_sb = qk_pool.tile([P, H * NSPAN], DT, tag="k_sb")
                v_sb = qk_pool.tile([P, H * NSPAN], DT, tag="v_sb")
                vdec = qk_pool.tile([P, H * NSPAN], DT, tag="vdec")

                def evict_q(h, ps):
                    # q_raw = psum + bq ; q_dec = (psum + bq) * qdec
                    nc.vector.tensor_scalar_add(q_raw[:, h * NSPAN:(h + 1) * NSPAN], ps[:], bqk_sb[:, 2 * h:2 * h + 1])
                    nc.vector.scalar_tensor_tensor(
                        q_dec[:, h * NSPAN:(h + 1) * NSPAN], ps[:], bqk_sb[:, 2 * h:2 * h + 1],
                        qdec_sb[:, h * NSPAN:(h + 1) * NSPAN], op0=add, op1=mult,
                    )

                def evict_k(h, ps):
                    nc.scalar.activation(
                        k_sb[:, h * NSPAN:(h + 1) * NSPAN], ps[:],
                        mybir.ActivationFunctionType.Identity, bias=bqk_sb[:, 2 * h + 1:2 * h + 2],
                    )

                if it == 0:
                    # span 0: kt-outer across 4 banks (3 qkv + 1 borrowed from
                    # the idle outproj pool) so each arriving x/w granule feeds
                    # 4 matmuls at once; then per-head tails stagger evictions.
                    for wi, (W_slice, evict) in enumerate(((wq_slice, evict_q), (wk_slice, evict_k))):
                        ps_h = [qkv_ps.tile([P, NSPAN], F32, tag="qkvps", name=f"s0ps{wi}_{h}")
                                for h in range(3)]
                        ps_h.append(out_ps.tile([P, NSPAN], F32, tag="outps", name=f"s0ps{wi}_3"))
                        for kt in range(8):
                            for h in range(H):
                                nc.tensor.matmul(
                                    ps_h[h][:], lhsT=W_slice(kt, h), rhs=xts(kt),
                                    start=(kt == 0), stop=False,
                                )
                        for h in range(H):
                            for kt in range(8, KT):
                                nc.tensor.matmul(
                                    ps_h[h][:], lhsT=W_slice(kt, h), rhs=xts(kt),
                                    start=False, stop=(kt == KT - 1),
                                )
                            evict(h, ps_h[h])
                else:
                    for h in range(H):
                        ps = qkv_ps.tile([P, NSPAN], F32, tag="qkvps")
                        for kt in range(KT):
                            nc.tensor.matmul(
                                ps[:],
                                lhsT=wq_slice(kt, h),
                                rhs=xts(kt),
                                start=(kt == 0), stop=(kt == KT - 1),
                            )
                        evict_q(h, ps)
                        ps = qkv_ps.tile([P, NSPAN], F32, tag="qkvps")
                        for kt in range(KT):
                            nc.tensor.matmul(
                                ps[:],
                                lhsT=wk_slice(kt, h),
                                rhs=xts(kt),
                                start=(kt == 0), stop=(kt == KT - 1),
                            )
                        evict_k(h, ps)

                for ns in range(4):
                    ps = qkv_ps.tile([P, NSPAN], F32, tag="qkvps")
                    for kt in range(KT):
                        nc.tensor.matmul(
                            ps[:],
                            lhsT=xts(kt)[:, ns * P:(ns + 1) * P],
                            rhs=wv_q[kt // 4][:, (kt % 4) * 512:(kt % 4 + 1) * 512],
                            start=(kt == 0), stop=(kt == KT - 1),
                        )
                    nc.vector.tensor_tensor(v_sb[:, ns * 512:(ns + 1) * 512], ps[:], bvf_sb[:], op=add)
                    nc.vector.tensor_tensor(vdec[:, ns * 512:(ns + 1) * 512], v_sb[:, ns * 512:(ns + 1) * 512], kdecv_sb[:], op=mult)


                # ---- attention + output projection, chunk by chunk ----
                # In the last span there is no next-span qkv work to hide the
                # serial S-chain, so hoist every chunk's scoresT/kT (which only
                # need k_sb/q_raw) ahead of the chain.
                last_span = (it == NIT - 1)
                scT_l = [None] * 4
                kT_l = [None] * 4

                def mk_sckt(p, need_kt=True):
                    # scoresT for all 4 heads into one psum bank. In the last
                    # span the qkv PSUM pool (3 banks) is idle, so the hoisted
                    # sc/kT tiles rotate through it instead of serializing on
                    # the single attn-sc bank.
                    ps_pool = qkv_ps if last_span else attn_sc_ps
                    ps_tag = "qkvps" if last_span else "sc"
                    sc_ps = ps_pool.tile([P, 512], F32, tag=ps_tag, name=f"sc_ps{p}")
                    for h in range(H):
                        nc.tensor.matmul(
                            sc_ps[:, h * L:(h + 1) * L],
                            lhsT=k_sb[:, h * NSPAN + p * L: h * NSPAN + (p + 1) * L],
                            rhs=q_raw[:, h * NSPAN + p * L: h * NSPAN + (p + 1) * L],
                            start=True, stop=True,
                        )
                    scT = sc_pool.tile([P, 512], DT, tag="scT", name=f"scT{p}")
                    nc.vector.tensor_tensor(scT[:], sc_ps[:], maskt_sb[:], op=mult)

                    if not need_kt:
                        scT_l[p] = scT
                        return
                    # kT (transpose k chunk) for all 4 heads
                    kt_ps = ps_pool.tile([P, 512], DT, tag=ps_tag, name=f"kt_ps{p}")
                    for h in range(H):
                        nc.tensor.transpose(
                            kt_ps[:, h * HD:(h + 1) * HD],
                            k_sb[:, h * NSPAN + p * L: h * NSPAN + (p + 1) * L],
                            ident[:],
                        )
                    kT = sc_pool.tile([P, 512], DT, tag="kT", name=f"kT{p}")
                    nc.scalar.copy(kT[:], kt_ps[:])
                    scT_l[p], kT_l[p] = scT, kT

                if last_span:
                    for p in range(4):
                        # the global last chunk's S update is dead work
                        mk_sckt(p, need_kt=(p < 3))
                for p in range(4):
                    if not last_span:
                        mk_sckt(p)
                    scT, kT = scT_l[p], kT_l[p]

                    # o = v^T @ scoresT + S^T @ qdec   [e, i] per head
                    o_ps = attn_o_ps.tile([P, 512], F32, tag="o")
                    for h in range(H):
                        nc.tensor.matmul(
                            o_ps[:, h * L:(h + 1) * L],
                            lhsT=v_sb[:, p * 512 + h * HD: p * 512 + (h + 1) * HD],
                            rhs=scT[:, h * L:(h + 1) * L],
                            start=True, stop=False,
                        )
                        nc.tensor.matmul(
                            o_ps[:, h * L:(h + 1) * L],
                            lhsT=S_bf[:, h * HD:(h + 1) * HD],
                            rhs=q_dec[:, h * NSPAN + p * L: h * NSPAN + (p + 1) * L],
                            start=False, stop=True,
                        )
                    # evict o per head so the out-projection's first
                    # accumulation matmul (h=0) isn't gated on the full CAST
                    ob = ob_pool.tile([P, 512], DT, tag="ob")
                    for h in range(H):
                        nc.vector.tensor_copy(ob[:, h * L:(h + 1) * L],
                                              o_ps[:, h * L:(h + 1) * L])

                    if not (last_span and p == 3):
                        # S <- S * blockdecay + kT^T @ vdec (fused per head,
                        # bf16; dead work for the global last chunk)
                        su_ps = attn_o_ps.tile([P, 512], F32, tag="o", name="su_ps")
                        for h in range(H):
                            nc.tensor.matmul(
                                su_ps[:, h * HD:(h + 1) * HD],
                                lhsT=kT[:, h * HD:(h + 1) * HD],
                                rhs=vdec[:, p * 512 + h * HD: p * 512 + (h + 1) * HD],
                                start=True, stop=True,
                            )
                        for h in range(H):
                            nc.vector.scalar_tensor_tensor(
                                S_bf[:, h * HD:(h + 1) * HD], S_bf[:, h * HD:(h + 1) * HD],
                                bdfc_sb[:, h:h + 1], su_ps[:, h * HD:(h + 1) * HD],
                                op0=mult, op1=add,
                            )

                    # output projection for this chunk's 128 tokens
                    split_dma = last_span and p == 3
                    outb = outb_pool.tile([P, C], DT, tag="outb")
                    for ct in range(4):
                        ops = out_ps.tile([P, 512], F32, tag="outps")
                        for h in range(H):
                            nc.tensor.matmul(
                                ops[:],
                                lhsT=ob[:, h * L:(h + 1) * L],
                                rhs=wo_sb[:, h * C + ct * 512: h * C + (ct + 1) * 512],
                                start=(h == 0), stop=(h == H - 1),
                            )
                        if ct % 2 == 0:
                            nc.vector.tensor_copy(outb[:, ct * 512:(ct + 1) * 512], ops[:])
                        else:
                            nc.scalar.copy(outb[:, ct * 512:(ct + 1) * 512], ops[:])
                        if split_dma and ct == 1:
                            nc.sync.dma_start(outd[n0 + p * L: n0 + (p + 1) * L, 0:1024],
                                              outb[:, 0:1024])
                        if split_dma and ct == 2:
                            nc.sync.dma_start(outd[n0 + p * L: n0 + (p + 1) * L, 1024:1536],
                                              outb[:, 1024:1536])
                    if split_dma:
                        nc.sync.dma_start(outd[n0 + p * L: n0 + (p + 1) * L, 1536:2048],
                                          outb[:, 1536:2048])
                    else:
                        nc.sync.dma_start(outd[n0 + p * L: n0 + (p + 1) * L, :], outb[:])

    nc.compile()
    return nc


def _host_inputs(x, Wqkv, bqkv, Wout, bout, slopes):
    """Per-core input maps (numpy, host-side sharding + packing)."""
    in_maps = []
    # packed transpose of x, shared by the 4 cores of each batch:
    # xd[c, it, kt, n] = x[b, it*512+n, kt*128+c]  (contiguous DMA granules)
    _xtp_cache = [
        np.ascontiguousarray(
            x[b].astype(BF16).reshape(NIT, NSPAN, KT, P).transpose(3, 0, 2, 1)
        )
        for b in range(B)
    ]
    i = np.arange(L, dtype=np.float64)
    for core in range(8):
        b, g = core // 4, core % 4
        h0 = 4 * g
        hsel = slice(h0 * HD, (h0 + H) * HD)

        xb = _xtp_cache[b]

        def pack_w(Wslice):
            # (C, 512) -> [c_in_tile(128), kt*512 + col]
            return np.ascontiguousarray(
                Wslice.astype(BF16).reshape(KT, P, H * HD).transpose(1, 0, 2).reshape(P, KT * 512)
            )

        wq = pack_w(Wqkv[:, 0 * C:1 * C][:, hsel])
        wk = pack_w(Wqkv[:, 1 * C:2 * C][:, hsel])
        wv = pack_w(Wqkv[:, 2 * C:3 * C][:, hsel])
        # Wout rows for these heads: [d(128), h*2048 + outc]
        wo = np.ascontiguousarray(
            Wout[hsel, :].astype(BF16).reshape(H, HD, C).transpose(1, 0, 2).reshape(P, H * C)
        )

        s = slopes[h0:h0 + H].astype(np.float64)  # (4,)
        diffT = (i[None, :] - i[:, None])          # [j, i] = i - j
        maskt = np.concatenate(
            [np.where(diffT >= 0, np.exp(-s[h] * diffT), 0.0) for h in range(H)],
            axis=1,
        ).astype(np.float32)                       # [128, 4*128]
        qdec_l = [np.exp(-s[h] * i) for h in range(H)]        # each (L,)
        qdec = np.concatenate(
            [np.broadcast_to(np.tile(qdec_l[h], NSPAN // L)[None, :], (P, NSPAN)) for h in range(H)],
            axis=1,
        ).astype(np.float32)                       # [128, 4*512]
        kdecv = np.concatenate(
            [np.broadcast_to(np.exp(-s[h] * (L - i))[:, None], (P, HD)) for h in range(H)],
            axis=1,
        ).astype(np.float32)                       # [128, 4*128]
        bdfc = np.broadcast_to(
            np.exp(-s * L)[None, :], (P, H)
        ).astype(np.float32)                       # [128, 4]
        # per-head, per-partition(d) q/k biases: columns [bq_h0, bk_h0, bq_h1, ...]
        bq_heads = bqkv[0 * C:1 * C][hsel].reshape(H, HD)
        bk_heads = bqkv[1 * C:2 * C][hsel].reshape(H, HD)
        bqk = np.zeros((P, 2 * H), dtype=np.float32)
        for h in range(H):
            bqk[:, 2 * h] = bq_heads[h]
            bqk[:, 2 * h + 1] = bk_heads[h]
        bvf = np.broadcast_to(bqkv[2 * C:3 * C][hsel][None, :], (P, H * HD)).astype(np.float32)

        in_maps.append({
            "x": xb, "wq": wq, "wk": wk, "wv": wv, "wo": wo,
            "maskt": maskt, "qdec": qdec, "kdecv": kdecv,
            "bdfc": np.ascontiguousarray(bdfc),
            "bqk": bqk, "bvf": np.ascontiguousarray(bvf),
        })
    return in_maps


def kernel(x, Wqkv, bqkv, Wout, bout, slopes, _want_trace=False):
    from concourse import bass_utils

    x = np.asarray(x, dtype=np.float32)
    Wqkv = np.asarray(Wqkv, dtype=np.float32)
    bqkv = np.asarray(bqkv, dtype=np.float32)
    Wout = np.asarray(Wout, dtype=np.float32)
    bout = np.asarray(bout, dtype=np.float32)
    slopes = np.asarray(slopes, dtype=np.float32)

    if "nc" not in _CACHE:
        _CACHE["nc"] = _build()
    nc = _CACHE["nc"]

    in_maps = _host_inputs(x, Wqkv, bqkv, Wout, bout, slopes)
    res = bass_utils.run_bass_kernel_spmd(
        nc, in_maps, core_ids=list(range(8)), trace=_want_trace,
    )
    out = np.zeros((B, N, C), dtype=np.float32)
    for core in range(8):
        out[core // 4] += res.results[core]["out"].astype(np.float32)
    out += bout[None, None, :]
    if _want_trace:
        _CACHE["last_result"] = res
    return out


# revision 14
# speedup vs baseline: 1.0043x; 1.0043x over previous
"""Trainium2 Bass kernel for MinimalLightningAttention2.

Strategy (8 NeuronCores, SPMD, no collectives):
  core c -> batch b = c // 4, head group g = c % 4 (heads 4g..4g+3).
  Each core computes, fully fused on-chip:
    qkv projection (its 4 heads' columns of Wqkv)
    chunked lightning-attention scan (L=128 chunks, per-head decay state S)
    row-parallel partial of the output projection (its 4 heads' rows of Wout)
  Host sums the 4 partial outputs per batch and adds bout.

Layouts on device (per core):
  xT   [c, it, kt, n]  host-pre-transposed/packed bf16 x; every DMA granule
                       is contiguous per partition (kt-major within a span)
  q,k  [d,   n]  (lhsT = Wq/Wk tile, rhs = xT)
  v    [n, h*d]  (lhsT = xT tile,    rhs = Wv)
  attn output oT [e, i] per head -> directly the lhsT of the Wout matmul.
All matmuls in bf16 (PSUM accumulation fp32); decay masks applied in fp32
during PSUM eviction; decay state S kept in bf16 with a fused per-head
(S*blockdecay + update) scalar_tensor_tensor.

Perf notes (measured on trn2 via NTFF traces):
  - kernel is PE-streaming-bound (~95% TensorMatrix busy); remaining time
    is startup (time-to-first-MM), the output-DMA tail, and the serial
    S-chain bubbles in the last span.
  - startup: ~100 warmup matmuls on the identity tile run while the first
    DMAs land, so the PE's HAM clock-gate is already at full rate when
    real work starts; first x/wq k-tiles are 128KB each, in need-order on
    the sync HWDGE ring; span-0 q/k run kt-outer across 4 PSUM banks so
    each arriving granule immediately feeds 4 matmuls.
  - outputs are bf16 and go per-chunk on the sync HWDGE ring: hardware
    descriptor generation (the gpsimd SWDGE takes ~4.2us/chunk to build
    descriptors, which serialized into a ~11us drain tail).
  - PSUM banks: qkv 3 + attn-sc 1 + attn-o 2 + outproj 2 = 8 (all used;
    span-0's 4th projection bank is borrowed from the idle outproj pool).
"""

import math

import numpy as np
import ml_dtypes

B, N, C = 2, 4096, 2048
H_TOT = 16
HD = 128          # head dim
H = 4             # heads per core
L = 128           # attention chunk length
NCH = N // L      # 32 chunks
KT = C // 128     # 16 contraction tiles for the projections
NSPAN = 512       # tokens per outer iteration
NIT = N // NSPAN  # 8 outer iterations
P = 128

BF16 = ml_dtypes.bfloat16

_CACHE = {}


def _build():
    """Build + compile the SPMD Bass program (same program on all 8 cores)."""
    from contextlib import ExitStack

    import concourse.bass as bass
    import concourse.tile as tile
    from concourse import bacc, mybir

    DT = mybir.dt.bfloat16
    F32 = mybir.dt.float32

    nc = bacc.Bacc(
        "TRN2",
        target_bir_lowering=False,
        debug=False,
        enable_asserts=False,
        num_devices=8,
    )

    # host-packed transpose of x: xd[c, it, kt, n] = x[it*512+n, kt*128+c]
    # (per-partition contiguous within a span: kt-major, 1KB per kt)
    xd = nc.dram_tensor("x", [P, NIT, KT, NSPAN], DT, kind="ExternalInput").ap()
    # host-packed: [c, kt*512 + col] (col = head*128 + d), fully contiguous rows
    wqd = nc.dram_tensor("wq", [P, KT * 512], DT, kind="ExternalInput").ap()
    wkd = nc.dram_tensor("wk", [P, KT * 512], DT, kind="ExternalInput").ap()
    wvd = nc.dram_tensor("wv", [P, KT * 512], DT, kind="ExternalInput").ap()
    # host-packed: [d, h*2048 + outc]
    wod = nc.dram_tensor("wo", [P, H * C], DT, kind="ExternalInput").ap()
    masktd = nc.dram_tensor("maskt", [P, H * L], F32, kind="ExternalInput").ap()
    qdecd = nc.dram_tensor("qdec", [P, H * NSPAN], F32, kind="ExternalInput").ap()
    kdecvd = nc.dram_tensor("kdecv", [P, H * HD], F32, kind="ExternalInput").ap()
    bdfcd = nc.dram_tensor("bdfc", [P, H], F32, kind="ExternalInput").ap()
    bqkd = nc.dram_tensor("bqk", [P, 2 * H], F32, kind="ExternalInput").ap()
    bvfd = nc.dram_tensor("bvf", [P, H * HD], F32, kind="ExternalInput").ap()
    outd = nc.dram_tensor("out", [N, C], DT, kind="ExternalOutput").ap()

    mult = mybir.AluOpType.mult
    add = mybir.AluOpType.add

    with tile.TileContext(nc) as tc:
        with ExitStack() as ctx:
            const = ctx.enter_context(tc.tile_pool(name="const", bufs=1))
            xt0_pool = ctx.enter_context(tc.tile_pool(name="xt0", bufs=1))
            xt_pool = ctx.enter_context(tc.tile_pool(name="xt", bufs=2))
            qk_pool = ctx.enter_context(tc.tile_pool(name="qk", bufs=2))
            sc_pool = ctx.enter_context(tc.tile_pool(name="sc", bufs=4))
            ob_pool = ctx.enter_context(tc.tile_pool(name="ob", bufs=3))
            outb_pool = ctx.enter_context(tc.tile_pool(name="outb", bufs=2))
            qkv_ps = ctx.enter_context(tc.tile_pool(name="qkvps", bufs=3, space="PSUM"))
            attn_sc_ps = ctx.enter_context(tc.tile_pool(name="attnscps", bufs=1, space="PSUM"))
            attn_o_ps = ctx.enter_context(tc.tile_pool(name="attnops", bufs=2, space="PSUM"))
            out_ps = ctx.enter_context(tc.tile_pool(name="outps", bufs=2, space="PSUM"))

            # identity + PE warmup first: ~100 no-dep matmuls run while the
            # first DMAs land so the HAM clock-gate reaches full rate before
            # real work (and the PE never sits >3.4us idle before it).
            ident = const.tile([P, P], DT)
            from concourse.masks import make_identity
            make_identity(nc, ident)
            warm_ps = attn_o_ps.tile([P, 512], F32, tag="o", name="warm")
            for _ in range(100):
                nc.tensor.matmul(warm_ps[:, 0:P], lhsT=ident[:], rhs=ident[:],
                                 start=True, stop=True)

            # ---- constants / weights resident in SBUF ----
            # All big startup loads on the ONE sync ring, in need-order: each
            # DGE ring is a FIFO and rings fair-share HBM at packet
            # granularity. The first x k-tile and first wq k-tile are 128KB
            # each and interleave 1-ktile-wise so the PE can start after
            # ~256KB of DMA. Small decay/bias constants go on the gpsimd
            # (SWDGE) ring which flows earlier anyway.
            # startup needs few triggers: each sync dma_start is a ~650ns
            # DIRECT2D sequencer instruction, FIFO-serialized.
            xk0 = xt0_pool.tile([P, NSPAN], DT, name="xk0")          # span-0 kt0
            nc.sync.dma_start(xk0[:], xd[:, 0, 0, :])
            wqk0 = const.tile([P, 512], DT, name="wqk0")             # wq kt0
            nc.sync.dma_start(wqk0[:], wqd[:, 0:512])
            xkr = xt0_pool.tile([P, 3, NSPAN], DT, name="xkr")       # span-0 kt1-3
            nc.sync.dma_start(xkr[:], xd[:, 0, 1:4, :])
            wqkr = const.tile([P, 3 * 512], DT, name="wqkr")         # wq kt1-3
            nc.sync.dma_start(wqkr[:], wqd[:, 512:2048])
            xq0 = [None]  # span-0 x, quarters 1..3
            wq_q = [None]  # wq quarters 1..3
            for qq in range(1, 4):
                xq = xt_pool.tile([P, 4, NSPAN], DT, tag=f"xtq{qq}", name=f"xt0q{qq}")
                nc.sync.dma_start(xq[:], xd[:, 0, qq * 4:(qq + 1) * 4, :])
                xq0.append(xq)
                wt = const.tile([P, 4 * 512], DT, tag=f"wqq{qq}", name=f"wqq{qq}")
                nc.sync.dma_start(wt[:], wqd[:, qq * 2048:(qq + 1) * 2048])
                wq_q.append(wt)
            # small decay/bias constants (needed from the first evictions on,
            # ~26us in); everything on the one sync ring in need-order — the
            # gpsimd SWDGE is left with no DMA work at all.
            bqk_sb = const.tile([P, 2 * H], F32)
            nc.sync.dma_start(bqk_sb[:], bqkd[:])
            qdec_sb = const.tile([P, H * NSPAN], F32)
            nc.sync.dma_start(qdec_sb[:], qdecd[:])
            kdecv_sb = const.tile([P, H * HD], F32)
            nc.sync.dma_start(kdecv_sb[:], kdecvd[:])
            bdfc_sb = const.tile([P, H], F32)
            nc.sync.dma_start(bdfc_sb[:], bdfcd[:])
            bvf_sb = const.tile([P, H * HD], F32)
            nc.sync.dma_start(bvf_sb[:], bvfd[:])
            maskt_sb = const.tile([P, H * L], F32)
            nc.sync.dma_start(maskt_sb[:], masktd[:])
            wk_q = []
            wv_q = []
            for qq in range(4):
                t = const.tile([P, 4 * 512], DT, tag=f"wkq{qq}", name=f"wkq{qq}")
                nc.sync.dma_start(t[:], wkd[:, qq * 2048:(qq + 1) * 2048])
                wk_q.append(t)
            for qq in range(4):
                t = const.tile([P, 4 * 512], DT, tag=f"wvq{qq}", name=f"wvq{qq}")
                nc.sync.dma_start(t[:], wvd[:, qq * 2048:(qq + 1) * 2048])
                wv_q.append(t)
            wo_sb = const.tile([P, H * C], DT)
            nc.sync.dma_start(wo_sb[:], wod[:])

            def wq_slice(kt, h):
                if kt == 0:
                    return wqk0[:, h * HD:(h + 1) * HD]
                if kt < 4:
                    return wqkr[:, (kt - 1) * 512 + h * HD:(kt - 1) * 512 + (h + 1) * HD]
                return wq_q[kt // 4][:, (kt % 4) * 512 + h * HD:(kt % 4) * 512 + (h + 1) * HD]

            def wk_slice(kt, h):
                return wk_q[kt // 4][:, (kt % 4) * 512 + h * HD:(kt % 4) * 512 + (h + 1) * HD]

            # per-head decay state S [d, e], 4 heads side by side, bf16
            S_bf = const.tile([P, H * HD], DT)
            nc.vector.memset(S_bf[:], 0.0)

            xt_tiles = [None] * NIT
            for it in range(NIT):
                n0 = it * NSPAN
                # prefetch next span's xT (host-packed, four contiguous quarters)
                if it + 1 < NIT:
                    quarters = []
                    for qq in range(4):
                        xq = xt_pool.tile([P, 4, NSPAN], DT, tag=f"xtq{qq}", name=f"xtq{qq}_{it + 1}")
                        nc.sync.dma_start(xq[:], xd[:, it + 1, qq * 4:(qq + 1) * 4, :])
                        quarters.append(xq)
                    xt_tiles[it + 1] = quarters

                if it == 0:
                    def xts(kt):
                        if kt == 0:
                            return xk0[:]
                        if kt < 4:
                            return xkr[:, kt - 1, :]
                        return xq0[kt // 4][:, kt % 4, :]
                else:
                    xtab = xt_tiles[it]

                    def xts(kt, xtab=xtab):
                        return xtab[kt // 4][:, kt % 4, :]

                # ---- qkv projection for the span ----
                q_raw = qk_pool.tile([P, H * NSPAN], DT, tag="q_raw")
                q_dec = qk_pool.tile([P, H * NSPAN], DT, tag="q_dec")
                k_sb = qk_pool.tile([P, H * NSPAN], DT, tag="k_sb")
                v_sb = qk_pool.tile([P, H * NSPAN], DT, tag="v_sb")
                vdec = qk_pool.tile([P, H * NSPAN], DT, tag="vdec")

                def evict_q(h, ps):
                    # q_raw = psum + bq ; q_dec = (psum + bq) * qdec
                    nc.vector.tensor_scalar_add(q_raw[:, h * NSPAN:(h + 1) * NSPAN], ps[:], bqk_sb[:, 2 * h:2 * h + 1])
                    nc.vector.scalar_tensor_tensor(
                        q_dec[:, h * NSPAN:(h + 1) * NSPAN], ps[:], bqk_sb[:, 2 * h:2 * h + 1],
                        qdec_sb[:, h * NSPAN:(h + 1) * NSPAN], op0=add, op1=mult,
                    )

                def evict_k(h, ps):
                    nc.scalar.activation(
                        k_sb[:, h * NSPAN:(h + 1) * NSPAN], ps[:],
                        mybir.ActivationFunctionType.Identity, bias=bqk_sb[:, 2 * h + 1:2 * h + 2],
                    )

                if it == 0:
                    # span 0: kt-outer across 4 banks (3 qkv + 1 borrowed from
                    # the idle outproj pool) so each arriving x/w granule feeds
                    # 4 matmuls at once; then per-head tails stagger evictions.
                    for wi, (W_slice, evict) in enumerate(((wq_slice, evict_q), (wk_slice, evict_k))):
                        ps_h = [qkv_ps.tile([P, NSPAN], F32, tag="qkvps", name=f"s0ps{wi}_{h}")
                                for h in range(3)]
                        ps_h.append(out_ps.tile([P, NSPAN], F32, tag="outps", name=f"s0ps{wi}_3"))
                        for kt in range(8):
                            for h in range(H):
                                nc.tensor.matmul(
                                    ps_h[h][:], lhsT=W_slice(kt, h), rhs=xts(kt),
                                    start=(kt == 0), stop=False,
                                )
                        for h in range(H):
                            for kt in range(8, KT):
                                nc.tensor.matmul(
                                    ps_h[h][:], lhsT=W_slice(kt, h), rhs=xts(kt),
                                    start=False, stop=(kt == KT - 1),
                                )
                            evict(h, ps_h[h])
                else:
                    for h in range(H):
                        ps = qkv_ps.tile([P, NSPAN], F32, tag="qkvps")
                        for kt in range(KT):
                            nc.tensor.matmul(
                                ps[:],
                                lhsT=wq_slice(kt, h),
                                rhs=xts(kt),
                                start=(kt == 0), stop=(kt == KT - 1),
                            )
                        evict_q(h, ps)
                        ps = qkv_ps.tile([P, NSPAN], F32, tag="qkvps")
                        for kt in range(KT):
                            nc.tensor.matmul(
                                ps[:],
                                lhsT=wk_slice(kt, h),
                                rhs=xts(kt),
                                start=(kt == 0), stop=(kt == KT - 1),
                            )
                        evict_k(h, ps)

                for ns in range(4):
                    ps = qkv_ps.tile([P, NSPAN], F32, tag="qkvps")
                    for kt in range(KT):
                        nc.tensor.matmul(
                            ps[:],
                            lhsT=xts(kt)[:, ns * P:(ns + 1) * P],
                            rhs=wv_q[kt // 4][:, (kt % 4) * 512:(kt % 4 + 1) * 512],
                            start=(kt == 0), stop=(kt == KT - 1),
                        )
                    nc.vector.tensor_tensor(v_sb[:, ns * 512:(ns + 1) * 512], ps[:], bvf_sb[:], op=add)
                    nc.vector.tensor_tensor(vdec[:, ns * 512:(ns + 1) * 512], v_sb[:, ns * 512:(ns + 1) * 512], kdecv_sb[:], op=mult)


                # ---- attention + output projection, chunk by chunk ----
                # In the last span there is no next-span qkv work to hide the
                # serial S-chain, so hoist every chunk's scoresT/kT (which only
                # need k_sb/q_raw) ahead of the chain.
                last_span = (it == NIT - 1)
                scT_l = [None] * 4
                kT_l = [None] * 4

                def mk_sckt(p, need_kt=True):
                    # scoresT for all 4 heads into one psum bank. In the last
                    # span the qkv PSUM pool (3 banks) is idle, so the hoisted
                    # sc/kT tiles rotate through it instead of serializing on
                    # the single attn-sc bank.
                    ps_pool = qkv_ps if last_span else attn_sc_ps
                    ps_tag = "qkvps" if last_span else "sc"
                    sc_ps = ps_pool.tile([P, 512], F32, tag=ps_tag, name=f"sc_ps{p}")
                    for h in range(H):
                        nc.tensor.matmul(
                            sc_ps[:, h * L:(h + 1) * L],
                            lhsT=k_sb[:, h * NSPAN + p * L: h * NSPAN + (p + 1) * L],
                            rhs=q_raw[:, h * NSPAN + p * L: h * NSPAN + (p + 1) * L],
                            start=True, stop=True,
                        )
                    scT = sc_pool.tile([P, 512], DT, tag="scT", name=f"scT{p}")
                    nc.vector.tensor_tensor(scT[:], sc_ps[:], maskt_sb[:], op=mult)

                    if not need_kt:
                        scT_l[p] = scT
                        return
                    # kT (transpose k chunk) for all 4 heads
                    kt_ps = ps_pool.tile([P, 512], DT, tag=ps_tag, name=f"kt_ps{p}")
                    for h in range(H):
                        nc.tensor.transpose(
                            kt_ps[:, h * HD:(h + 1) * HD],
                            k_sb[:, h * NSPAN + p * L: h * NSPAN + (p + 1) * L],
                            ident[:],
                        )
                    kT = sc_pool.tile([P, 512], DT, tag="kT", name=f"kT{p}")
                    nc.scalar.copy(kT[:], kt_ps[:])
                    scT_l[p], kT_l[p] = scT, kT

                if last_span:
                    for p in range(4):
                        # the global last chunk's S update is dead work
                        mk_sckt(p, need_kt=(p < 3))
                for p in range(4):
                    if not last_span:
                        mk_sckt(p)
                    scT, kT = scT_l[p], kT_l[p]

                    # o = v^T @ scoresT + S^T @ qdec   [e, i] per head
                    o_ps = attn_o_ps.tile([P, 512], F32, tag="o")
                    for h in range(H):
                        nc.tensor.matmul(
                            o_ps[:, h * L:(h + 1) * L],
                            lhsT=v_sb[:, p * 512 + h * HD: p * 512 + (h + 1) * HD],
                            rhs=scT[:, h * L:(h + 1) * L],
                            start=True, stop=False,
                        )
                        nc.tensor.matmul(
                            o_ps[:, h * L:(h + 1) * L],
                            lhsT=S_bf[:, h * HD:(h + 1) * HD],
                            rhs=q_dec[:, h * NSPAN + p * L: h * NSPAN + (p + 1) * L],
                            start=False, stop=True,
                        )
                    # evict o per head so the out-projection's first
                    # accumulation matmul (h=0) isn't gated on the full CAST
                    ob = ob_pool.tile([P, 512], DT, tag="ob")
                    for h in range(H):
                        nc.vector.tensor_copy(ob[:, h * L:(h + 1) * L],
                                              o_ps[:, h * L:(h + 1) * L])

                    if not (last_span and p == 3):
                        # S <- S * blockdecay + kT^T @ vdec (fused per head,
                        # bf16; dead work for the global last chunk)
                        su_ps = attn_o_ps.tile([P, 512], F32, tag="o", name="su_ps")
                        for h in range(H):
                            nc.tensor.matmul(
                                su_ps[:, h * HD:(h + 1) * HD],
                                lhsT=kT[:, h * HD:(h + 1) * HD],
                                rhs=vdec[:, p * 512 + h * HD: p * 512 + (h + 1) * HD],
                                start=True, stop=True,
                            )
                        for h in range(H):
                            nc.vector.scalar_tensor_tensor(
                                S_bf[:, h * HD:(h + 1) * HD], S_bf[:, h * HD:(h + 1) * HD],
                                bdfc_sb[:, h:h + 1], su_ps[:, h * HD:(h + 1) * HD],
                                op0=mult, op1=add,
                            )

                    # output projection for this chunk's 128 tokens
                    split_dma = last_span and p == 3
                    outb = outb_pool.tile([P, C], DT, tag="outb")
                    for ct in range(4):
                        ops = out_ps.tile([P, 512], F32, tag="outps")
                        for h in range(H):
                            nc.tensor.matmul(
                                ops[:],
                                lhsT=ob[:, h * L:(h + 1) * L],
                                rhs=wo_sb[:, h * C + ct * 512: h * C + (ct + 1) * 512],
                                start=(h == 0), stop=(h == H - 1),
                            )
                        if ct % 2 == 0:
                            nc.vector.tensor_copy(outb[:, ct * 512:(ct + 1) * 512], ops[:])
                        else:
                            nc.scalar.copy(outb[:, ct * 512:(ct + 1) * 512], ops[:])
                        if split_dma and ct == 1:
                            nc.sync.dma_start(outd[n0 + p * L: n0 + (p + 1) * L, 0:1024],
                                              outb[:, 0:1024])
                        if split_dma and ct == 2:
                            nc.sync.dma_start(outd[n0 + p * L: n0 + (p + 1) * L, 1024:1536],
                                              outb[:, 1024:1536])
                    if split_dma:
                        nc.sync.dma_start(outd[n0 + p * L: n0 + (p + 1) * L, 1536:2048],
                                          outb[:, 1536:2048])
                    else:
                        nc.sync.dma_start(outd[n0 + p * L: n0 + (p + 1) * L, :], outb[:])

    nc.compile()
    return nc


def _host_inputs(x, Wqkv, bqkv, Wout, bout, slopes):
    """Per-core input maps (numpy, host-side sharding + packing)."""
    in_maps = []
    # packed transpose of x, shared by the 4 cores of each batch:
    # xd[c, it, kt, n] = x[b, it*512+n, kt*128+c]  (contiguous DMA granules)
    _xtp_cache = [
        np.ascontiguousarray(
            x[b].astype(BF16).reshape(NIT, NSPAN, KT, P).transpose(3, 0, 2, 1)
        )
        for b in range(B)
    ]
    i = np.arange(L, dtype=np.float64)
    for core in range(8):
        b, g = core // 4, core % 4
        h0 = 4 * g
        hsel = slice(h0 * HD, (h0 + H) * HD)

        xb = _xtp_cache[b]

        def pack_w(Wslice):
            # (C, 512) -> [c_in_tile(128), kt*512 + col]
            return np.ascontiguousarray(
                Wslice.astype(BF16).reshape(KT, P, H * HD).transpose(1, 0, 2).reshape(P, KT * 512)
            )

        wq = pack_w(Wqkv[:, 0 * C:1 * C][:, hsel])
        wk = pack_w(Wqkv[:, 1 * C:2 * C][:, hsel])
        wv = pack_w(Wqkv[:, 2 * C:3 * C][:, hsel])
        # Wout rows for these heads: [d(128), h*2048 + outc]
        wo = np.ascontiguousarray(
            Wout[hsel, :].astype(BF16).reshape(H, HD, C).transpose(1, 0, 2).reshape(P, H * C)
        )

        s = slopes[h0:h0 + H].astype(np.float64)  # (4,)
        diffT = (i[None, :] - i[:, None])          # [j, i] = i - j
        maskt = np.concatenate(
            [np.where(diffT >= 0, np.exp(-s[h] * diffT), 0.0) for h in range(H)],
            axis=1,
        ).astype(np.float32)                       # [128, 4*128]
        qdec_l = [np.exp(-s[h] * i) for h in range(H)]        # each (L,)
        qdec = np.concatenate(
            [np.broadcast_to(np.tile(qdec_l[h], NSPAN // L)[None, :], (P, NSPAN)) for h in range(H)],
            axis=1,
        ).astype(np.float32)                       # [128, 4*512]
        kdecv = np.concatenate(
            [np.broadcast_to(np.exp(-s[h] * (L - i))[:, None], (P, HD)) for h in range(H)],
            axis=1,
        ).astype(np.float32)                       # [128, 4*128]
        bdfc = np.broadcast_to(
            np.exp(-s * L)[None, :], (P, H)
        ).astype(np.float32)                       # [128, 4]
        # per-head, per-partition(d) q/k biases: columns [bq_h0, bk_h0, bq_h1, ...]
        bq_heads = bqkv[0 * C:1 * C][hsel].reshape(H, HD)
        bk_heads = bqkv[1 * C:2 * C][hsel].reshape(H, HD)
        bqk = np.zeros((P, 2 * H), dtype=np.float32)
        for h in range(H):
            bqk[:, 2 * h] = bq_heads[h]
            bqk[:, 2 * h + 1] = bk_heads[h]
        bvf = np.broadcast_to(bqkv[2 * C:3 * C][hsel][None, :], (P, H * HD)).astype(np.float32)

        in_maps.append({
            "x": xb, "wq": wq, "wk": wk, "wv": wv, "wo": wo,
            "maskt": maskt, "qdec": qdec, "kdecv": kdecv,
            "bdfc": np.ascontiguousarray(bdfc),
            "bqk": bqk, "bvf": np.ascontiguousarray(bvf),
        })
    return in_maps


def kernel(x, Wqkv, bqkv, Wout, bout, slopes, _want_trace=False):
    from concourse import bass_utils

    x = np.asarray(x, dtype=np.float32)
    Wqkv = np.asarray(Wqkv, dtype=np.float32)
    bqkv = np.asarray(bqkv, dtype=np.float32)
    Wout = np.asarray(Wout, dtype=np.float32)
    bout = np.asarray(bout, dtype=np.float32)
    slopes = np.asarray(slopes, dtype=np.float32)

    if "nc" not in _CACHE:
        _CACHE["nc"] = _build()
    nc = _CACHE["nc"]

    in_maps = _host_inputs(x, Wqkv, bqkv, Wout, bout, slopes)
    res = bass_utils.run_bass_kernel_spmd(
        nc, in_maps, core_ids=list(range(8)), trace=_want_trace,
    )
    out = np.zeros((B, N, C), dtype=np.float32)
    for core in range(8):
        out[core // 4] += res.results[core]["out"].astype(np.float32)
    out += bout[None, None, :]
    if _want_trace:
        _CACHE["last_result"] = res
    return out
